# revision 8
# baseline (speedup 1.0000x reference)
"""GCN layer (message passing + linear + ReLU) on 8 Trainium2 NeuronCores, v2.

out = relu(((scatter_add(h[src] -> dst) + x) * dis) @ W.T),
h = x * dis,  dis = rsqrt(deg + 1),  deg = in-degree via dst counts.

v2 strategy (vs v1): no device-side h table at all. The host passes x twice:
f32 shard rows (for the finalize +x) and a bf16 copy of the full x split in
two halves (gather tables). Edges are partitioned by dst owner and sorted by
dst; per 128-edge chunk the kernel gathers x_bf16[src] rows with SWDGE
dma_gather, scales them by dis[src] (per-edge, from a host-provided int16
deg[src] stream -> rsqrt on device), and accumulates S.T @ G into the dst
window's PSUM tile via one-hot bf16 matmuls. Finalize per window:
(+x, *dis_dst), PE transpose, 128x128 linear, ReLU, store.
"""
import numpy as np
import ml_dtypes

from concourse import bacc, bass, mybir, tile
from concourse.bass_utils import run_bass_kernel_spmd

F32 = mybir.dt.float32
BF16 = mybir.dt.bfloat16
I32 = mybir.dt.int32
I16 = mybir.dt.int16
AF = mybir.ActivationFunctionType
OP = mybir.AluOpType

N = 50000
E = 600000
D = 128
C = 8                      # cores
NPC = N // C               # 6250 nodes per core
WPC = (NPC + 127) // 128   # 49 windows per core
NPAD = WPC * 128           # 6272 padded shard rows
SPLIT = 32768              # src table split for int16 gather indices
NHI = N - SPLIT            # 17232
PASS_BOUNDS = [(0, SPLIT), (SPLIT, N)]
GB = 8                     # chunks per dma_gather batch (1024 idxs max)


# ---------------------------------------------------------------- host prep
def host_prep(edge_index):
    src = np.asarray(edge_index[0], dtype=np.int64)
    dst = np.asarray(edge_index[1], dtype=np.int64)
    order = np.argsort(dst, kind="stable")
    ss_all = src[order]
    dd_all = dst[order]
    counts = np.bincount(dst, minlength=N)
    deg_all = counts.astype(np.int64)  # in-degree per node (as dst)
    rowptr = np.zeros(N + 1, np.int64)
    rowptr[1:] = np.cumsum(counts)

    per_core = []
    need = np.zeros((C, 2, WPC), np.int64)
    for c in range(C):
        e0, e1 = rowptr[c * NPC], rowptr[(c + 1) * NPC]
        ss, dd = ss_all[e0:e1], dd_all[e0:e1]
        per_core.append((ss, dd))
        for p, (lo, hi) in enumerate(PASS_BOUNDS):
            m = (ss >= lo) & (ss < hi)
            w = (dd[m] - c * NPC) // 128
            need[c, p] = np.bincount(w, minlength=WPC)
    K = np.ceil(need.max(axis=0) / 128).astype(np.int64)  # [2, WPC]
    CH = K.sum(axis=1)
    cstart = np.zeros((2, WPC), np.int64)
    for p in range(2):
        cstart[p, 1:] = np.cumsum(K[p][:-1])

    cores = []
    for c in range(C):
        ss, dd = per_core[c]
        d = {}
        for p, (lo, hi) in enumerate(PASS_BOUNDS):
            m = (ss >= lo) & (ss < hi)
            sp = (ss[m] - lo).astype(np.int64)
            dg = deg_all[ss[m]].astype(np.int64)  # deg of the source node
            dloc = dd[m] - c * NPC
            w = dloc // 128
            g = np.zeros(CH[p] * 128, np.int64)
            s = np.full(CH[p] * 128, 255, np.int64)
            dgs = np.zeros(CH[p] * 128, np.int64)
            cnt = np.bincount(w, minlength=WPC)
            ofs = np.zeros(WPC, np.int64)
            ofs[1:] = np.cumsum(cnt[:-1])
            pos = cstart[p, w] * 128 + (np.arange(len(sp)) - ofs[w])
            g[pos] = sp
            s[pos] = dloc - w * 128
            dgs[pos] = dg
            tag = "lo" if p == 0 else "hi"
            # gather idx layout [128, CH*8]: stream pos j at [j%16, j//16],
            # replicated across the 8 groups of 16 partitions.
            d[f"gidx_{tag}"] = np.tile(
                g.reshape(-1, 16).T.astype(np.int16), (8, 1)
            ).copy()
            # slot / deg layout [128, CH]: stream pos j at [j%128, j//128]
            d[f"slots_{tag}"] = s.reshape(-1, 128).T.astype(np.int16).copy()
            d[f"degs_{tag}"] = dgs.reshape(-1, 128).T.astype(np.int16).copy()
        n0 = c * NPC
        rpv = np.full(NPAD + 1, rowptr[min((c + 1) * NPC, N)], np.int64)
        rpv[: NPC + 1] = rowptr[n0 : n0 + NPC + 1]
        d["rp0s"] = rpv[:NPAD].reshape(WPC, 128).T.astype(np.int32).copy()
        d["rp1s"] = rpv[1 : NPAD + 1].reshape(WPC, 128).T.astype(np.int32).copy()
        cores.append(d)
    return dict(K=K, CH=CH, cores=cores)


# ---------------------------------------------------------------- program
NQ = 4  # SWDGE queues; desc-gen for different queues runs concurrently


def build_program(K):
    K = np.asarray(K)
    CH = K.sum(axis=1)
    nc = bacc.Bacc(
        None, target_bir_lowering=False, debug=False, num_swdge_queues=NQ
    )

    xb_lo_p = nc.dram_tensor("xb_lo", [SPLIT, D], BF16, kind="ExternalInput")
    xb_hi_p = nc.dram_tensor("xb_hi", [NHI, D], BF16, kind="ExternalInput")
    xs_p = nc.dram_tensor("xs", [NPAD, D], F32, kind="ExternalInput")
    wt_p = nc.dram_tensor("wt", [D, D], F32, kind="ExternalInput")
    iotab_p = nc.dram_tensor("iotab", [128, 128], BF16, kind="ExternalInput")
    ident_p = nc.dram_tensor("ident", [128, 128], F32, kind="ExternalInput")
    rp0s_p = nc.dram_tensor("rp0s", [128, WPC], I32, kind="ExternalInput")
    rp1s_p = nc.dram_tensor("rp1s", [128, WPC], I32, kind="ExternalInput")
    gidx_p = [
        nc.dram_tensor("gidx_lo", [128, int(CH[0]) * 8], I16, kind="ExternalInput"),
        nc.dram_tensor("gidx_hi", [128, int(CH[1]) * 8], I16, kind="ExternalInput"),
    ]
    slots_p = [
        nc.dram_tensor("slots_lo", [128, int(CH[0])], I16, kind="ExternalInput"),
        nc.dram_tensor("slots_hi", [128, int(CH[1])], I16, kind="ExternalInput"),
    ]
    degs_p = [
        nc.dram_tensor("degs_lo", [128, int(CH[0])], I16, kind="ExternalInput"),
        nc.dram_tensor("degs_hi", [128, int(CH[1])], I16, kind="ExternalInput"),
    ]
    out_p = nc.dram_tensor("out", [NPAD, D], F32, kind="ExternalOutput")
    tables = [xb_lo_p, xb_hi_p]

    with tile.TileContext(nc) as tc:
        with (
            tc.tile_pool(name="const", bufs=1) as cpool,
            tc.tile_pool(name="gather", bufs=8) as gpool,
            tc.tile_pool(name="meta", bufs=2) as mpool,
            tc.tile_pool(name="sel", bufs=8) as spool,
            tc.tile_pool(name="fin", bufs=3) as fpool,
            tc.tile_pool(name="psA", bufs=2, space="PSUM") as psA,
            tc.tile_pool(name="psT", bufs=2, space="PSUM") as psT,
            tc.tile_pool(name="psO", bufs=2, space="PSUM") as psO,
        ):
            # --- constants
            wt_sb = cpool.tile([128, 128], F32, tag="wt")
            nc.sync.dma_start(wt_sb[:], wt_p[:])
            iota_sb = cpool.tile([128, 128], BF16, tag="iota")
            nc.sync.dma_start(iota_sb[:], iotab_p[:])
            ident_sb = cpool.tile([128, 128], F32, tag="ident")
            nc.sync.dma_start(ident_sb[:], ident_p[:])

            # --- prefetch gather indices, slot ids, per-edge src degrees
            gidx_sb, stf, disrc = [], [], []
            for p in range(2):
                gi = cpool.tile([128, int(CH[p]) * 8], I16, tag=f"gidx{p}")
                nc.sync.dma_start(gi[:], gidx_p[p][:])
                si = mpool.tile([128, int(CH[p])], I16, tag="si")
                nc.sync.dma_start(si[:], slots_p[p][:])
                sf = cpool.tile([128, int(CH[p])], BF16, tag=f"sf{p}")
                nc.vector.tensor_copy(sf[:], si[:])
                di = mpool.tile([128, int(CH[p])], I16, tag="di")
                nc.sync.dma_start(di[:], degs_p[p][:])
                # dis_src = rsqrt(deg+1) in bf16 [128, CH]
                df = cpool.tile([128, int(CH[p])], F32, tag=f"df{p}")
                nc.vector.tensor_copy(df[:], di[:])
                nc.vector.tensor_scalar_add(out=df[:], in0=df[:], scalar1=1.0)
                rc = cpool.tile([128, int(CH[p])], F32, tag=f"rc{p}")
                nc.vector.reciprocal_approx_fast(rc[:], df[:])
                ds = cpool.tile([128, int(CH[p])], F32, tag=f"ds{p}")
                nc.scalar.activation(ds[:], rc[:], AF.Sqrt)
                db = cpool.tile([128, int(CH[p])], BF16, tag=f"db{p}")
                nc.vector.tensor_copy(db[:], ds[:])
                gidx_sb.append(gi)
                stf.append(sf)
                disrc.append(db)

            xs_v = xs_p[:].rearrange("(u p) d -> p u d", p=128)
            xsw = cpool.tile([128, NPAD], F32, tag="xsw")
            nc.sync.dma_start(
                out=xsw[:].rearrange("p (u e) -> p u e", e=128), in_=xs_v[:, :, :]
            )

            # --- dis_dst = 1/sqrt(deg+1) from rowptr diffs, [128, WPC]
            r0i = cpool.tile([128, WPC], I32, tag="r0i")
            nc.sync.dma_start(r0i[:], rp0s_p[:])
            r1i = cpool.tile([128, WPC], I32, tag="r1i")
            nc.sync.dma_start(r1i[:], rp1s_p[:])
            r0f = cpool.tile([128, WPC], F32, tag="r0f")
            nc.vector.tensor_copy(r0f[:], r0i[:])
            r1f = cpool.tile([128, WPC], F32, tag="r1f")
            nc.vector.tensor_copy(r1f[:], r1i[:])
            dgt = cpool.tile([128, WPC], F32, tag="dgt")
            nc.vector.tensor_tensor(out=dgt[:], in0=r1f[:], in1=r0f[:], op=OP.subtract)
            nc.vector.tensor_scalar_add(out=dgt[:], in0=dgt[:], scalar1=1.0)
            rcs = cpool.tile([128, WPC], F32, tag="rcs")
            nc.vector.reciprocal(rcs[:], dgt[:])
            dis_s = cpool.tile([128, WPC], F32, tag="dis_s")
            nc.scalar.activation(dis_s[:], rcs[:], AF.Sqrt)

            # --- aggregation: each window visited once, consuming chunks from
            # both pass streams (hi table chunks then lo) into one PSUM tile.
            agg_sb = cpool.tile([128, NPAD], F32, tag="agg")
            gq = 0  # round-robin SWDGE queue counter
            pos = [0, 0]          # per-pass chunk position
            cur = [None, None]    # per-pass current (gt, Sw, B) batch tiles

            def pull_batch(p):
                nch = int(CH[p])
                b0 = pos[p]
                B = min(GB, nch - b0)
                gt = gpool.tile([128, GB * 128], BF16, tag="gt")
                gv = gt[:, : B * 128].rearrange("p (b e) -> p b e", e=128)
                nonlocal gq
                nc.gpsimd.dma_gather(
                    gv, tables[p][:], gidx_sb[p][:, b0 * 8 : (b0 + B) * 8],
                    B * 128, B * 128, 128, queue_num=gq % NQ,
                )
                gq += 1
                # scale gathered rows by dis[src] (per lane, bcast along feat)
                dsl = disrc[p][:, b0 : b0 + B]
                in1 = bass.AP(dsl.tensor, dsl.offset, list(dsl.ap) + [[0, 128]])
                nc.vector.tensor_tensor(
                    out=gt[:, : B * 128].rearrange("p (b e) -> p b e", e=128),
                    in0=gv, in1=in1, op=OP.mult,
                )
                # one-hot S
                Sw = spool.tile([128, GB * 128], BF16, tag="S")
                base = stf[p][:, b0 : b0 + B]
                in0 = bass.AP(base.tensor, base.offset, list(base.ap) + [[0, 128]])
                ii = iota_sb[:]
                in1b = bass.AP(ii.tensor, ii.offset, [ii.ap[0], [0, B], ii.ap[1]])
                nc.vector.tensor_tensor(
                    out=Sw[:, : B * 128].rearrange("p (b e) -> p b e", e=128),
                    in0=in0, in1=in1b, op=OP.is_equal,
                )
                cur[p] = (gt, Sw, b0)

            for u in range(WPC):
                Ktot = int(K[1][u]) + int(K[0][u])
                sl = slice(u * 128, (u + 1) * 128)
                if Ktot == 0:
                    nc.vector.tensor_copy(agg_sb[:, sl], xsw[:, sl])
                    nc.scalar.activation(
                        agg_sb[:, sl], agg_sb[:, sl], AF.Copy,
                        scale=dis_s[:, u : u + 1])
                    continue
                ps = psA.tile([128, 128], F32, tag="pacc")
                kin = 0
                for p in (1, 0):
                    for _ in range(int(K[p][u])):
                        if cur[p] is None or pos[p] - cur[p][2] >= GB:
                            pull_batch(p)
                        gt, Sw, b0 = cur[p]
                        kk = pos[p] - b0
                        pos[p] += 1
                        nc.tensor.matmul(
                            ps[:],
                            lhsT=Sw[:, kk * 128 : (kk + 1) * 128],
                            rhs=gt[:, kk * 128 : (kk + 1) * 128],
                            start=(kin == 0),
                            stop=(kin == Ktot - 1),
                        )
                        kin += 1
                nc.vector.tensor_tensor(
                    out=agg_sb[:, sl], in0=ps[:], in1=xsw[:, sl], op=OP.add)
                nc.scalar.activation(
                    agg_sb[:, sl], agg_sb[:, sl], AF.Copy,
                    scale=dis_s[:, u : u + 1])

            # --- finalize phase: transpose + linear + relu + store
            out_v = out_p[:].rearrange("(u p) d -> p u d", p=128)
            for u in range(WPC):
                sl = slice(u * 128, (u + 1) * 128)
                pt = psT.tile([128, 128], F32, tag="pt")
                nc.tensor.transpose(pt[:], agg_sb[:, sl], ident_sb[:])
                att = fpool.tile([128, 128], F32, tag="fat")
                nc.scalar.copy(att[:], pt[:])
                po = psO.tile([128, 128], F32, tag="po")
                nc.tensor.matmul(po[:], lhsT=att[:], rhs=wt_sb[:], start=True, stop=True)
                ot = fpool.tile([128, 128], F32, tag="fo")
                nc.scalar.activation(ot[:], po[:], AF.Relu)
                nc.sync.dma_start(out_v[:, u, :], ot[:])

    nc.compile()
    return nc


# ---------------------------------------------------------------- runner
_CACHE = {}


def _get_program(K):
    key = K.tobytes()
    if key not in _CACHE:
        _CACHE[key] = build_program(K)
    return _CACHE[key]


def make_in_maps(x, W, prep):
    x = np.asarray(x, np.float32)
    Wt = np.ascontiguousarray(np.asarray(W, np.float32).T)
    xb = x.astype(ml_dtypes.bfloat16)
    xb_lo = np.ascontiguousarray(xb[:SPLIT])
    xb_hi = np.ascontiguousarray(xb[SPLIT:])
    iotab = np.tile(
        np.arange(128, dtype=np.float32)[None, :], (128, 1)
    ).astype(ml_dtypes.bfloat16)
    ident = np.eye(128, dtype=np.float32)
    in_maps = []
    for c in range(C):
        cd = prep["cores"][c]
        xs = np.zeros((NPAD, D), np.float32)
        xs[:NPC] = x[c * NPC : (c + 1) * NPC]
        in_maps.append(
            {
                "xb_lo": xb_lo,
                "xb_hi": xb_hi,
                "xs": xs,
                "wt": Wt,
                "iotab": iotab,
                "ident": ident,
                "rp0s": cd["rp0s"],
                "rp1s": cd["rp1s"],
                "gidx_lo": cd["gidx_lo"],
                "gidx_hi": cd["gidx_hi"],
                "slots_lo": cd["slots_lo"],
                "slots_hi": cd["slots_hi"],
                "degs_lo": cd["degs_lo"],
                "degs_hi": cd["degs_hi"],
            }
        )
    return in_maps


def run_spmd(x, edge_index, W, trace=False, **spmd_kwargs):
    prep = host_prep(edge_index)
    nc = _get_program(prep["K"])
    in_maps = make_in_maps(x, W, prep)
    res = run_bass_kernel_spmd(nc, in_maps, list(range(C)), trace=trace, **spmd_kwargs)
    out = np.concatenate([res.results[c]["out"][:NPC] for c in range(C)], axis=0)
    return out.astype(np.float32), res


def kernel(x, edge_index, N=None, W=None, **_):
    out, _res = run_spmd(np.asarray(x), np.asarray(edge_index), np.asarray(W))
    return out


# revision 12
# speedup vs baseline: 1.0250x; 1.0250x over previous
"""GCN layer (message passing + linear + ReLU) on 8 Trainium2 NeuronCores, v2.

out = relu(((scatter_add(h[src] -> dst) + x) * dis) @ W.T),
h = x * dis,  dis = rsqrt(deg + 1),  deg = in-degree via dst counts.

v2 strategy (vs v1): no device-side h table at all. The host passes x twice:
f32 shard rows (for the finalize +x) and a bf16 copy of the full x split in
two halves (gather tables). Edges are partitioned by dst owner and sorted by
dst; per 128-edge chunk the kernel gathers x_bf16[src] rows with SWDGE
dma_gather, scales them by dis[src] (per-edge, from a host-provided int16
deg[src] stream -> rsqrt on device), and accumulates S.T @ G into the dst
window's PSUM tile via one-hot bf16 matmuls. Finalize per window:
(+x, *dis_dst), PE transpose, 128x128 linear, ReLU, store.
"""
import numpy as np
import ml_dtypes

from concourse import bacc, bass, mybir, tile
from concourse.bass_utils import run_bass_kernel_spmd

F32 = mybir.dt.float32
BF16 = mybir.dt.bfloat16
I32 = mybir.dt.int32
I16 = mybir.dt.int16
AF = mybir.ActivationFunctionType
OP = mybir.AluOpType

N = 50000
E = 600000
D = 128
C = 8                      # cores
NPC = N // C               # 6250 nodes per core
WPC = (NPC + 127) // 128   # 49 windows per core
NPAD = WPC * 128           # 6272 padded shard rows
SPLIT = 32768              # src table split for int16 gather indices
NHI = N - SPLIT            # 17232
PASS_BOUNDS = [(0, SPLIT), (SPLIT, N)]
GB = 8                     # chunks per dma_gather batch (1024 idxs max)


# ---------------------------------------------------------------- host prep
def host_prep(edge_index):
    src = np.asarray(edge_index[0], dtype=np.int64)
    dst = np.asarray(edge_index[1], dtype=np.int64)
    order = np.argsort(dst, kind="stable")
    ss_all = src[order]
    dd_all = dst[order]
    counts = np.bincount(dst, minlength=N)
    deg_all = counts.astype(np.int64)  # in-degree per node (as dst)
    rowptr = np.zeros(N + 1, np.int64)
    rowptr[1:] = np.cumsum(counts)

    per_core = []
    need = np.zeros((C, 2, WPC), np.int64)
    for c in range(C):
        e0, e1 = rowptr[c * NPC], rowptr[(c + 1) * NPC]
        ss, dd = ss_all[e0:e1], dd_all[e0:e1]
        per_core.append((ss, dd))
        for p, (lo, hi) in enumerate(PASS_BOUNDS):
            m = (ss >= lo) & (ss < hi)
            w = (dd[m] - c * NPC) // 128
            need[c, p] = np.bincount(w, minlength=WPC)
    K = np.ceil(need.max(axis=0) / 128).astype(np.int64)  # [2, WPC]
    CH = K.sum(axis=1)
    cstart = np.zeros((2, WPC), np.int64)
    for p in range(2):
        cstart[p, 1:] = np.cumsum(K[p][:-1])

    cores = []
    for c in range(C):
        ss, dd = per_core[c]
        d = {}
        for p, (lo, hi) in enumerate(PASS_BOUNDS):
            m = (ss >= lo) & (ss < hi)
            sp = (ss[m] - lo).astype(np.int64)
            dg = deg_all[ss[m]].astype(np.int64)  # deg of the source node
            dloc = dd[m] - c * NPC
            w = dloc // 128
            g = np.zeros(CH[p] * 128, np.int64)
            s = np.full(CH[p] * 128, 255, np.int64)
            dgs = np.zeros(CH[p] * 128, np.int64)
            cnt = np.bincount(w, minlength=WPC)
            ofs = np.zeros(WPC, np.int64)
            ofs[1:] = np.cumsum(cnt[:-1])
            pos = cstart[p, w] * 128 + (np.arange(len(sp)) - ofs[w])
            g[pos] = sp
            s[pos] = dloc - w * 128
            dgs[pos] = dg
            tag = "lo" if p == 0 else "hi"
            # gather idx layout [128, CH*8]: stream pos j at [j%16, j//16],
            # replicated across the 8 groups of 16 partitions.
            d[f"gidx_{tag}"] = np.tile(
                g.reshape(-1, 16).T.astype(np.int16), (8, 1)
            ).copy()
            # slot / deg layout [128, CH]: stream pos j at [j%128, j//128]
            d[f"slots_{tag}"] = s.reshape(-1, 128).T.astype(np.int16).copy()
            d[f"degs_{tag}"] = dgs.reshape(-1, 128).T.astype(np.int16).copy()
        n0 = c * NPC
        rpv = np.full(NPAD + 1, rowptr[min((c + 1) * NPC, N)], np.int64)
        rpv[: NPC + 1] = rowptr[n0 : n0 + NPC + 1]
        d["rp0s"] = rpv[:NPAD].reshape(WPC, 128).T.astype(np.int32).copy()
        d["rp1s"] = rpv[1 : NPAD + 1].reshape(WPC, 128).T.astype(np.int32).copy()
        cores.append(d)
    return dict(K=K, CH=CH, cores=cores)


# ---------------------------------------------------------------- program
NQ = 4  # SWDGE queues; desc-gen for different queues runs concurrently


def build_program(K):
    K = np.asarray(K)
    CH = K.sum(axis=1)
    nc = bacc.Bacc(
        None, target_bir_lowering=False, debug=False, num_swdge_queues=NQ
    )

    xb_lo_p = nc.dram_tensor("xb_lo", [SPLIT, D], BF16, kind="ExternalInput")
    xb_hi_p = nc.dram_tensor("xb_hi", [NHI, D], BF16, kind="ExternalInput")
    xs_p = nc.dram_tensor("xs", [NPAD, D], F32, kind="ExternalInput")
    wt_p = nc.dram_tensor("wt", [D, D], F32, kind="ExternalInput")
    iotab_p = nc.dram_tensor("iotab", [128, 128], BF16, kind="ExternalInput")
    ident_p = nc.dram_tensor("ident", [128, 128], F32, kind="ExternalInput")
    rp0s_p = nc.dram_tensor("rp0s", [128, WPC], I32, kind="ExternalInput")
    rp1s_p = nc.dram_tensor("rp1s", [128, WPC], I32, kind="ExternalInput")
    gidx_p = [
        nc.dram_tensor("gidx_lo", [128, int(CH[0]) * 8], I16, kind="ExternalInput"),
        nc.dram_tensor("gidx_hi", [128, int(CH[1]) * 8], I16, kind="ExternalInput"),
    ]
    slots_p = [
        nc.dram_tensor("slots_lo", [128, int(CH[0])], I16, kind="ExternalInput"),
        nc.dram_tensor("slots_hi", [128, int(CH[1])], I16, kind="ExternalInput"),
    ]
    degs_p = [
        nc.dram_tensor("degs_lo", [128, int(CH[0])], I16, kind="ExternalInput"),
        nc.dram_tensor("degs_hi", [128, int(CH[1])], I16, kind="ExternalInput"),
    ]
    out_p = nc.dram_tensor("out", [NPAD, D], F32, kind="ExternalOutput")
    tables = [xb_lo_p, xb_hi_p]

    with tile.TileContext(nc) as tc:
        with (
            tc.tile_pool(name="const", bufs=1) as cpool,
            tc.tile_pool(name="gather", bufs=8) as gpool,
            tc.tile_pool(name="meta", bufs=2) as mpool,
            tc.tile_pool(name="sel", bufs=8) as spool,
            tc.tile_pool(name="fin", bufs=3) as fpool,
            tc.tile_pool(name="psA", bufs=2, space="PSUM") as psA,
            tc.tile_pool(name="psT", bufs=2, space="PSUM") as psT,
            tc.tile_pool(name="psO", bufs=2, space="PSUM") as psO,
        ):
            # --- constants (iota first: needed by the first S build)
            iota_sb = cpool.tile([128, 128], BF16, tag="iota")
            nc.sync.dma_start(iota_sb[:], iotab_p[:])

            # --- prefetch gather indices, slot ids, per-edge src degrees.
            # Pass 1 (hi) streams first: windows consume hi chunks first, so
            # the first gather only waits on the hi-stream loads.
            gidx_sb, stf, disrc = [None, None], [None, None], [None, None]
            for p in (1, 0):
                gi = cpool.tile([128, int(CH[p]) * 8], I16, tag=f"gidx{p}")
                nc.sync.dma_start(gi[:], gidx_p[p][:])
                si = mpool.tile([128, int(CH[p])], I16, tag="si")
                nc.sync.dma_start(si[:], slots_p[p][:])
                sf = cpool.tile([128, int(CH[p])], BF16, tag=f"sf{p}")
                nc.vector.tensor_copy(sf[:], si[:])
                di = mpool.tile([128, int(CH[p])], I16, tag="di")
                nc.sync.dma_start(di[:], degs_p[p][:])
                # dis_src = rsqrt(deg+1) in bf16 [128, CH]
                df = cpool.tile([128, int(CH[p])], F32, tag=f"df{p}")
                nc.vector.tensor_copy(df[:], di[:])
                nc.vector.tensor_scalar_add(out=df[:], in0=df[:], scalar1=1.0)
                rc = cpool.tile([128, int(CH[p])], F32, tag=f"rc{p}")
                nc.vector.reciprocal_approx_fast(rc[:], df[:])
                ds = cpool.tile([128, int(CH[p])], F32, tag=f"ds{p}")
                nc.scalar.activation(ds[:], rc[:], AF.Sqrt)
                db = cpool.tile([128, int(CH[p])], BF16, tag=f"db{p}")
                nc.vector.tensor_copy(db[:], ds[:])
                gidx_sb[p] = gi
                stf[p] = sf
                disrc[p] = db

            # --- dis_dst = 1/sqrt(deg+1) from rowptr diffs, [128, WPC]
            r0i = cpool.tile([128, WPC], I32, tag="r0i")
            nc.sync.dma_start(r0i[:], rp0s_p[:])
            r1i = cpool.tile([128, WPC], I32, tag="r1i")
            nc.sync.dma_start(r1i[:], rp1s_p[:])
            r0f = cpool.tile([128, WPC], F32, tag="r0f")
            nc.vector.tensor_copy(r0f[:], r0i[:])
            r1f = cpool.tile([128, WPC], F32, tag="r1f")
            nc.vector.tensor_copy(r1f[:], r1i[:])
            dgt = cpool.tile([128, WPC], F32, tag="dgt")
            nc.vector.tensor_tensor(out=dgt[:], in0=r1f[:], in1=r0f[:], op=OP.subtract)
            nc.vector.tensor_scalar_add(out=dgt[:], in0=dgt[:], scalar1=1.0)
            rcs = cpool.tile([128, WPC], F32, tag="rcs")
            nc.vector.reciprocal_approx_fast(rcs[:], dgt[:])
            dis_s = cpool.tile([128, WPC], F32, tag="dis_s")
            nc.scalar.activation(dis_s[:], rcs[:], AF.Sqrt)

            # --- remaining constants + xs (needed from the first window end)
            wt_sb = cpool.tile([128, 128], F32, tag="wt")
            nc.sync.dma_start(wt_sb[:], wt_p[:])
            ident_sb = cpool.tile([128, 128], F32, tag="ident")
            nc.sync.dma_start(ident_sb[:], ident_p[:])
            xs_v = xs_p[:].rearrange("(u p) d -> p u d", p=128)
            xsw = cpool.tile([128, NPAD], F32, tag="xsw")
            nc.sync.dma_start(
                out=xsw[:].rearrange("p (u e) -> p u e", e=128), in_=xs_v[:, :, :]
            )

            # --- aggregation: each window visited once, consuming chunks from
            # both pass streams (hi table chunks then lo) into one PSUM tile.
            # Finalize (transpose + linear + relu + store) is inlined per
            # window so it overlaps the next windows' gathers.
            out_v = out_p[:].rearrange("(u p) d -> p u d", p=128)
            gq = 0  # round-robin SWDGE queue counter
            pos = [0, 0]          # per-pass chunk position
            cur = [None, None]    # per-pass current (gt, Sw, B) batch tiles

            def pull_batch(p):
                nch = int(CH[p])
                b0 = pos[p]
                B = min(GB, nch - b0)
                gt = gpool.tile([128, GB * 128], BF16, tag="gt")
                gv = gt[:, : B * 128].rearrange("p (b e) -> p b e", e=128)
                nonlocal gq
                nc.gpsimd.dma_gather(
                    gv, tables[p][:], gidx_sb[p][:, b0 * 8 : (b0 + B) * 8],
                    B * 128, B * 128, 128, queue_num=gq % NQ,
                )
                gq += 1
                # scale gathered rows by dis[src] (per lane, bcast along feat)
                dsl = disrc[p][:, b0 : b0 + B]
                in1 = bass.AP(dsl.tensor, dsl.offset, list(dsl.ap) + [[0, 128]])
                nc.vector.tensor_tensor(
                    out=gt[:, : B * 128].rearrange("p (b e) -> p b e", e=128),
                    in0=gv, in1=in1, op=OP.mult,
                )
                # one-hot S
                Sw = spool.tile([128, GB * 128], BF16, tag="S")
                base = stf[p][:, b0 : b0 + B]
                in0 = bass.AP(base.tensor, base.offset, list(base.ap) + [[0, 128]])
                ii = iota_sb[:]
                in1b = bass.AP(ii.tensor, ii.offset, [ii.ap[0], [0, B], ii.ap[1]])
                nc.vector.tensor_tensor(
                    out=Sw[:, : B * 128].rearrange("p (b e) -> p b e", e=128),
                    in0=in0, in1=in1b, op=OP.is_equal,
                )
                cur[p] = (gt, Sw, b0)

            for u in range(WPC):
                Ktot = int(K[1][u]) + int(K[0][u])
                sl = slice(u * 128, (u + 1) * 128)
                aw = fpool.tile([128, 128], F32, tag="aw")
                if Ktot == 0:
                    nc.vector.tensor_copy(aw[:], xsw[:, sl])
                else:
                    ps = psA.tile([128, 128], F32, tag="pacc")
                    kin = 0
                    for p in (1, 0):
                        for _ in range(int(K[p][u])):
                            if cur[p] is None or pos[p] - cur[p][2] >= GB:
                                pull_batch(p)
                            gt, Sw, b0 = cur[p]
                            kk = pos[p] - b0
                            pos[p] += 1
                            nc.tensor.matmul(
                                ps[:],
                                lhsT=Sw[:, kk * 128 : (kk + 1) * 128],
                                rhs=gt[:, kk * 128 : (kk + 1) * 128],
                                start=(kin == 0),
                                stop=(kin == Ktot - 1),
                            )
                            kin += 1
                    nc.vector.tensor_tensor(
                        out=aw[:], in0=ps[:], in1=xsw[:, sl], op=OP.add)
                # (+x) done above; now *dis_dst, transpose, linear, relu, store
                nc.scalar.activation(
                    aw[:], aw[:], AF.Copy, scale=dis_s[:, u : u + 1])
                pt = psT.tile([128, 128], F32, tag="pt")
                nc.tensor.transpose(pt[:], aw[:], ident_sb[:])
                att = fpool.tile([128, 128], F32, tag="fat")
                nc.scalar.copy(att[:], pt[:])
                po = psO.tile([128, 128], F32, tag="po")
                nc.tensor.matmul(po[:], lhsT=att[:], rhs=wt_sb[:], start=True, stop=True)
                ot = fpool.tile([128, 128], F32, tag="fo")
                nc.scalar.activation(ot[:], po[:], AF.Relu)
                nc.sync.dma_start(out_v[:, u, :], ot[:])

    nc.compile()
    return nc


# ---------------------------------------------------------------- runner
_CACHE = {}


def _get_program(K):
    key = K.tobytes()
    if key not in _CACHE:
        _CACHE[key] = build_program(K)
    return _CACHE[key]


def make_in_maps(x, W, prep):
    x = np.asarray(x, np.float32)
    Wt = np.ascontiguousarray(np.asarray(W, np.float32).T)
    xb = x.astype(ml_dtypes.bfloat16)
    xb_lo = np.ascontiguousarray(xb[:SPLIT])
    xb_hi = np.ascontiguousarray(xb[SPLIT:])
    iotab = np.tile(
        np.arange(128, dtype=np.float32)[None, :], (128, 1)
    ).astype(ml_dtypes.bfloat16)
    ident = np.eye(128, dtype=np.float32)
    in_maps = []
    for c in range(C):
        cd = prep["cores"][c]
        xs = np.zeros((NPAD, D), np.float32)
        xs[:NPC] = x[c * NPC : (c + 1) * NPC]
        in_maps.append(
            {
                "xb_lo": xb_lo,
                "xb_hi": xb_hi,
                "xs": xs,
                "wt": Wt,
                "iotab": iotab,
                "ident": ident,
                "rp0s": cd["rp0s"],
                "rp1s": cd["rp1s"],
                "gidx_lo": cd["gidx_lo"],
                "gidx_hi": cd["gidx_hi"],
                "slots_lo": cd["slots_lo"],
                "slots_hi": cd["slots_hi"],
                "degs_lo": cd["degs_lo"],
                "degs_hi": cd["degs_hi"],
            }
        )
    return in_maps


def run_spmd(x, edge_index, W, trace=False, **spmd_kwargs):
    prep = host_prep(edge_index)
    nc = _get_program(prep["K"])
    in_maps = make_in_maps(x, W, prep)
    res = run_bass_kernel_spmd(nc, in_maps, list(range(C)), trace=trace, **spmd_kwargs)
    out = np.concatenate([res.results[c]["out"][:NPC] for c in range(C)], axis=0)
    return out.astype(np.float32), res


def kernel(x, edge_index, N=None, W=None, **_):
    out, _res = run_spmd(np.asarray(x), np.asarray(edge_index), np.asarray(W))
    return out
